# revision 15
# baseline (speedup 1.0000x reference)
"""LocalOTLoss (masked Sinkhorn OT loss) Trainium2 Bass kernel.

Strategy (8 NeuronCores, pure data parallel over batch):
  Each core processes BP=64 batches: v[64,256,512], t[64,128,512] f32.

  Phase 1a (per batch): DMA v/t; ACT Square+accum row sumsq (stored for all
  batches); PE-transpose v/t (12 matmuls); cos-sim matmul psA = t_raw^T
  v_raw in f32r; stash raw psA into the resident X buffer.
  Boundary: ONE Sqrt pass over all 192 sumsq columns + DVE reciprocal
  (batching the Sqrt avoids per-batch ACT table reloads: Sqrt lives in a
  different HW act table than Exp; Square/Copy/Exp share one).
  Phase 1b (per batch): broadcast inv_v row (GpSimd partition_broadcast),
  DVE-fold inv_v, ACT Exp with per-partition inv_t/EPS scale -> X
  (layout [m=NT, b, n=NV+2], dust col NV = e^g, pad col = 0), DVE builds
  M = X*(1-A) in bf16, PE transposes X -> X2 (layout [n, b, c, m]) so the
  w-update becomes a pure matmul.

  Phase 2 (5 Sinkhorn iters, non-log domain): a = mu/(X b) via 64
  accumulating f32r matmuls with a block-one-hot stationary (stride-66
  slot trick, one shared slot buffer); b = nu/(X2^T a) via 128 matmuls
  over X2 (two n-chunks), dustbin handled analytically; transposes of the
  small [64,*] operands ride on the PE.

  Loss = a^T M b per batch via bf16 matmuls -> [64,1]; host averages.

Masks are all-ones in this workload (spec fill=ones); a numpy fallback
handles any other mask pattern.

Toolchain workarounds for this environment:
  - TileContext's final Drain may carry >1 sem wait; this walrus build
    allows only 1 per TPB_CTRL Drain -> _drain_and_barrier patched.
  - Bacc defers register allocation to finalize(); nc.finalize() must run
    before serialization.
  - Custom-DVE ops (tensor_tensor_reduce etc.) crash the runtime here;
    only native DVE/ACT/PE ops are used.
  - f32r matmul operands must be produced by f32r-typed writes (DMA with
    both sides bitcast, or ACT/DVE ops with f32r out). Memset can't write
    f32r -> constants come from inline DRAM tensors.
  - f32r moving operands need an even free dim -> X rows padded to 258.
"""

import sys
import traceback

for _p in ("/opt/trn_rl_repo",):
    if _p not in sys.path:
        sys.path.insert(0, _p)

import numpy as np
import ml_dtypes

import concourse.bass as bass
import concourse.bacc as bacc
import concourse.tile as tile
from concourse import mybir
from concourse.bass_utils import run_bass_kernel_spmd

F32 = mybir.dt.float32
F32R = mybir.dt.float32r
BF16 = mybir.dt.bfloat16
AF = mybir.ActivationFunctionType
ALU = mybir.AluOpType

B, NV, NT, D = 512, 256, 128, 512
NCORES = 8
BP = B // NCORES  # 64 batches per core
EPS = 0.1
ITERS = 5

# effective marginals (mirror reference: exp(log(mu + 1e-9)))
MU_R = 1.0 / (NV + 1e-9) + 1e-9
NU_R = 1.0 / (NT + 1e-9) + 1e-9
DUST = 1.0 + 1e-9


def _install_drain_patch():
    import bass_rust
    from concourse.vector_clock import ScopedClock

    if getattr(tile.TileContext, "_drain_patch_installed", False):
        return

    def _patched_drain_and_barrier(self, tick_clock, wait_clock):
        drain_inst = self.nc.sync.drain()
        wait_clock.add_sem_waits(
            drain_inst.ins, ScopedClock({None: tick_clock.global_clock})
        )
        d = drain_inst.ins
        si = d.sync_info
        waits = list(si.on_wait) if si is not None else []
        if len(waits) > 1:
            si.on_wait = waits[:1]
            d.sync_info = si
            for w in waits[1:]:
                d2 = self.nc.sync.drain().ins
                d2.sync_info = bass_rust.SyncInfo(on_wait=[w], on_update=[])
        self.nc.all_engine_barrier()
        assert self.sems is not None
        popped = self.nc._tile_sem_poison_stack.pop()
        assert popped is self._sem_poison
        self.nc.clear_and_free_semaphores(list(self.sems.allocated().values()))
        self.nc.all_engine_barrier()

    tile.TileContext._drain_and_barrier = _patched_drain_and_barrier
    tile.TileContext._drain_patch_installed = True


_install_drain_patch()


def build_bass(eg: float, bp: int = BP) -> bass.Bass:
    """Build the per-core Bass module. eg = exp(gamma/eps)."""
    nc = bacc.Bacc(trn_type="TRN2")
    v = nc.dram_tensor("v", [bp, NV, D], F32, kind="ExternalInput")
    t = nc.dram_tensor("t", [bp, NT, D], F32, kind="ExternalInput")
    out = nc.dram_tensor("out", [bp, 1], F32, kind="ExternalOutput")
    dram = {
        "ident": nc.inline_tensor(np.eye(128, dtype=np.float32), name="ident"),
        "ones": nc.inline_tensor(np.ones((128, 128), np.float32), name="onesd"),
        "zeros_sl": nc.inline_tensor(
            np.zeros((128, 66 * bp), np.float32), name="zeros_sl"
        ),
        "zeros_bf": nc.inline_tensor(
            np.zeros((128, 66 * bp), ml_dtypes.bfloat16), name="zeros_bf"
        ),
        "egcol": nc.inline_tensor(
            np.concatenate(
                [
                    np.full((128, bp, 1), eg, np.float32),
                    np.zeros((128, bp, 1), np.float32),
                ],
                axis=2,
            ),
            name="egcol",
        ),
    }

    with tile.TileContext(nc) as tc:
        _body(nc, tc, v, t, out, dram, eg, bp)
    nc.finalize()
    return nc


def _body(nc, tc, v, t, out, dram, eg, bp):
    from contextlib import ExitStack

    NVP = NV + 2  # padded X row

    with ExitStack() as ctx:
        consts = ctx.enter_context(tc.tile_pool(name="consts", bufs=1))
        big = ctx.enter_context(tc.tile_pool(name="big", bufs=1))

        ident_sb = consts.tile([128, 128], F32)
        nc.sync.dma_start(out=ident_sb, in_=dram["ident"][:, :])
        ident_r = consts.tile([128, 128], F32R)
        nc.sync.dma_start(out=ident_r, in_=dram["ident"][:, :].bitcast(F32R))

        # Residents
        X_all = big.tile([128, bp, NVP], F32R)  # [m, b, n] raw A then X
        X2_all = big.tile([128, bp, 2, 128], F32R)  # [n, b, chunk, m]
        M_all = big.tile([128, bp, NV], BF16)  # [m, b, n]
        ssq_all = big.tile([128, 3, bp], F32)  # v0, v1, t sumsq
        inv3 = big.tile([128, 3, bp], F32)
        inv_t10 = big.tile([128, bp], F32)
        neg_it = big.tile([128, bp], F32)
        nc.sync.dma_start(
            out=X_all[:, :, NV : NV + 2],
            in_=dram["egcol"][:, :, :].bitcast(F32R),
        )

        # ---------------- Phase 1a ----------------
        with ExitStack() as p1:
            io = p1.enter_context(tc.tile_pool(name="io", bufs=3))
            work = p1.enter_context(tc.tile_pool(name="work", bufs=3))
            pvt = p1.enter_context(tc.tile_pool(name="pvt", bufs=1, space="PSUM"))
            pa = p1.enter_context(tc.tile_pool(name="pa", bufs=2, space="PSUM"))

            for b in range(bp):
                vt = io.tile([128, 2, D], F32, tag="vt")
                nc.sync.dma_start(
                    out=vt, in_=v[b].rearrange("(h p) d -> p h d", p=128)
                )
                tt = io.tile([128, D], F32, tag="tt")
                nc.sync.dma_start(out=tt, in_=t[b])

                # row sumsq on ACT (Square+accum; stays in exp-family table)
                sqd = work.tile([128, D], F32, tag="sqd")
                nc.scalar.activation(
                    out=sqd, in_=vt[:, 0, :], func=AF.Square,
                    accum_out=ssq_all[:, 0, b : b + 1],
                )
                nc.scalar.activation(
                    out=sqd, in_=vt[:, 1, :], func=AF.Square,
                    accum_out=ssq_all[:, 1, b : b + 1],
                )
                nc.scalar.activation(
                    out=sqd, in_=tt, func=AF.Square,
                    accum_out=ssq_all[:, 2, b : b + 1],
                )

                # transposes (raw data)
                psv0 = pvt.tile([128, 512], F32, tag="psv0")
                psv1 = pvt.tile([128, 512], F32, tag="psv1")
                pst = pvt.tile([128, 512], F32, tag="pst")
                for c in range(4):
                    pdst = psv0 if c < 2 else psv1
                    col0 = 256 * (c % 2)
                    for h in range(2):
                        nc.tensor.transpose(
                            out=pdst[:, col0 + 128 * h : col0 + 128 * (h + 1)],
                            in_=vt[:, h, 128 * c : 128 * (c + 1)],
                            identity=ident_sb,
                        )
                    nc.tensor.transpose(
                        out=pst[:, 128 * c : 128 * (c + 1)],
                        in_=tt[:, 128 * c : 128 * (c + 1)],
                        identity=ident_sb,
                    )

                vT = work.tile([128, 4, 256], F32R, tag="vT")
                nc.scalar.copy(
                    out=vT[:, 0:2, :].rearrange("p a b -> p (a b)"), in_=psv0
                )
                nc.vector.tensor_copy(
                    out=vT[:, 2:4, :].rearrange("p a b -> p (a b)"), in_=psv1
                )
                tT = work.tile([128, 512], F32R, tag="tT")
                nc.scalar.copy(out=tT, in_=pst)

                # cos-sim raw matmul
                psA = pa.tile([128, 256], F32, tag="psA")
                for c in range(4):
                    nc.tensor.matmul(
                        psA,
                        lhsT=tT[:, 128 * c : 128 * (c + 1)],
                        rhs=vT[:, c, :],
                        start=(c == 0),
                        stop=(c == 3),
                    )
                # stash raw A into X slot (f32r write)
                nc.vector.tensor_copy(out=X_all[:, b, 0:NV], in_=psA)

        # ---------------- boundary: batched rsqrt ----------------
        with tc.tile_pool(name="bnd", bufs=1) as bnd:
            rt3 = bnd.tile([128, 3, bp], F32)
            nc.scalar.activation(
                out=rt3.rearrange("p a b -> p (a b)"),
                in_=ssq_all.rearrange("p a b -> p (a b)"),
                func=AF.Sqrt,
            )
            nc.vector.reciprocal(
                out=inv3.rearrange("p a b -> p (a b)"),
                in_=rt3.rearrange("p a b -> p (a b)"),
            )
            nc.vector.tensor_scalar_mul(inv_t10, inv3[:, 2, :], 1.0 / EPS)
            nc.vector.tensor_scalar_mul(neg_it, inv3[:, 2, :], -1.0)

        # ---------------- Phase 1b ----------------
        with ExitStack() as p1b:
            wk = p1b.enter_context(tc.tile_pool(name="wk", bufs=3))
            pr = p1b.enter_context(tc.tile_pool(name="pr", bufs=2, space="PSUM"))
            px = p1b.enter_context(tc.tile_pool(name="px", bufs=2, space="PSUM"))

            for b in range(bp):
                psr = pr.tile([1, 256], F32, tag="psr")
                nc.tensor.transpose(
                    out=psr[:, 0:128], in_=inv3[:, 0, b : b + 1], identity=ident_sb
                )
                nc.tensor.transpose(
                    out=psr[:, 128:256], in_=inv3[:, 1, b : b + 1], identity=ident_sb
                )
                ivrow = wk.tile([1, 256], F32, tag="ivrow")
                nc.vector.tensor_copy(out=ivrow, in_=psr)
                ivb = wk.tile([128, 256], F32, tag="ivb")
                nc.gpsimd.partition_broadcast(ivb, ivrow)

                An = wk.tile([128, 256], F32, tag="An")
                nc.vector.tensor_mul(
                    out=An, in0=X_all[:, b, 0:NV].bitcast(F32), in1=ivb
                )
                # X = exp(An * inv_t / EPS)
                nc.scalar.activation(
                    out=X_all[:, b, 0:NV], in_=An, func=AF.Exp,
                    scale=inv_t10[:, b : b + 1],
                )
                # om = 1 - An*inv_t
                om = wk.tile([128, 256], F32, tag="om")
                nc.vector.tensor_scalar(
                    out=om, in0=An, scalar1=neg_it[:, b : b + 1], scalar2=1.0,
                    op0=ALU.mult, op1=ALU.add,
                )
                nc.vector.tensor_mul(
                    out=M_all[:, b, :], in0=om, in1=X_all[:, b, 0:NV].bitcast(F32)
                )
                # X2 = X^T (two 128-chunks)
                psX2 = px.tile([128, 256], F32R, tag="psX2")
                nc.tensor.transpose(
                    out=psX2[:, 0:128], in_=X_all[:, b, 0:128], identity=ident_r
                )
                nc.tensor.transpose(
                    out=psX2[:, 128:256], in_=X_all[:, b, 128:256], identity=ident_r
                )
                nc.scalar.copy(
                    out=X2_all[:, b, :, :].rearrange("p a b -> p (a b)"),
                    in_=psX2.bitcast(F32),
                )

        # ---------------- Phase 2: Sinkhorn ----------------
        with ExitStack() as p2:
            ph2 = p2.enter_context(tc.tile_pool(name="ph2", bufs=1))
            p2w = p2.enter_context(tc.tile_pool(name="p2w", bufs=3))
            pps = p2.enter_context(tc.tile_pool(name="pps", bufs=1, space="PSUM"))
            ppw = p2.enter_context(tc.tile_pool(name="ppw", bufs=1, space="PSUM"))
            ppt = p2.enter_context(tc.tile_pool(name="ppt", bufs=2, space="PSUM"))

            Bmat = ph2.tile([128, bp], F32R)
            nc.sync.dma_start(out=Bmat, in_=dram["ones"][:, 0:bp].bitcast(F32R))
            bdust = ph2.tile([bp, 1], F32)
            nc.vector.memset(bdust, 1.0)
            Sdiag = ph2.tile([128, 66 * bp], F32R)  # shared slot buffer
            nc.sync.dma_start(out=Sdiag, in_=dram["zeros_sl"][:, :].bitcast(F32R))
            Ldiag = ph2.tile([128, 66 * bp], BF16)
            nc.sync.dma_start(out=Ldiag, in_=dram["zeros_bf"][:, :].bitcast(BF16))
            Amat = ph2.tile([bp, NVP], F32R)

            sl_slots = bass.AP(
                tensor=Sdiag.tensor,
                offset=Sdiag.offset,
                ap=[list(Sdiag.ap[0]), [66, bp]],
            )
            lb_slots = bass.AP(
                tensor=Ldiag.tensor,
                offset=Ldiag.offset,
                ap=[list(Ldiag.ap[0]), [66, bp]],
            )

            for it in range(ITERS):
                # -- u-update: a = mu / (X b); dust col gives eg*sum(b) --
                nc.vector.tensor_copy(out=sl_slots, in_=Bmat.bitcast(F32))
                psS = pps.tile([bp, NVP], F32, tag="psS")
                for b in range(bp):
                    nc.tensor.matmul(
                        psS,
                        lhsT=Sdiag[:, 65 * b : 65 * b + bp],
                        rhs=X_all[:, b, :],
                        start=(b == 0),
                        stop=(b == bp - 1),
                    )
                bd_eg = p2w.tile([bp, 1], F32, tag="bd_eg")
                nc.vector.tensor_scalar_mul(bd_eg, bdust, eg)
                den = p2w.tile([bp, NVP], F32, tag="den")
                nc.vector.tensor_scalar(
                    out=den, in0=psS, scalar1=bd_eg, scalar2=None, op0=ALU.add
                )
                rec = p2w.tile([bp, NVP], F32, tag="rec")
                nc.vector.reciprocal(out=rec, in_=den)
                nc.vector.tensor_scalar_mul(Amat[:, 0:NV], rec[:, 0:NV], MU_R)
                nc.vector.tensor_scalar_mul(
                    Amat[:, NV:NVP], rec[:, NV:NVP], DUST
                )

                # -- w-update: b = nu / (X2 a + eg*a_dust) --
                psT = ppt.tile([128, 128], F32R, tag="psT")
                nc.tensor.transpose(
                    out=psT[:, 0:bp], in_=Amat[:, 0:128],
                    identity=ident_r[0:bp, 0:bp],
                )
                nc.tensor.transpose(
                    out=psT[:, bp : 2 * bp], in_=Amat[:, 128:256],
                    identity=ident_r[0:bp, 0:bp],
                )
                psW = ppw.tile([bp, 128], F32, tag="psW")
                for c in range(2):
                    nc.vector.tensor_copy(
                        out=sl_slots, in_=psT[:, bp * c : bp * (c + 1)].bitcast(F32)
                    )
                    for b in range(bp):
                        nc.tensor.matmul(
                            psW,
                            lhsT=Sdiag[:, 65 * b : 65 * b + bp],
                            rhs=X2_all[:, b, c, :],
                            start=(c == 0 and b == 0),
                            stop=(c == 1 and b == bp - 1),
                        )
                ad_eg = p2w.tile([bp, 1], F32, tag="ad_eg")
                nc.vector.tensor_scalar_mul(
                    ad_eg, Amat[:, NV : NV + 1].bitcast(F32), eg
                )
                wden = p2w.tile([bp, 128], F32, tag="wden")
                nc.vector.tensor_scalar(
                    out=wden, in0=psW, scalar1=ad_eg, scalar2=None, op0=ALU.add
                )
                wrec = p2w.tile([bp, 128], F32R, tag="wrec")
                with nc.allow_low_precision(reason="f32r feed for PE transpose"):
                    nc.vector.reciprocal(out=wrec, in_=wden)
                psB = ppt.tile([128, bp], F32R, tag="psB")
                nc.tensor.transpose(out=psB, in_=wrec, identity=ident_r[0:bp, 0:bp])
                nc.vector.tensor_scalar_mul(Bmat, psB.bitcast(F32), NU_R)

                # b_dust = DUST / (eg * sum_n a_total)
                sa = p2w.tile([bp, 1], F32, tag="sa")
                nc.vector.tensor_reduce(
                    out=sa, in_=Amat[:, 0 : NV + 1].bitcast(F32),
                    axis=mybir.AxisListType.X, op=ALU.add,
                )
                sa2 = p2w.tile([bp, 1], F32, tag="sa2")
                nc.vector.tensor_scalar_mul(sa2, sa, eg)
                sa3 = p2w.tile([bp, 1], F32, tag="sa3")
                nc.vector.reciprocal(out=sa3, in_=sa2)
                nc.vector.tensor_scalar_mul(bdust, sa3, DUST)

            # -- loss = a^T M b per batch (bf16 matmuls) --
            nc.vector.tensor_copy(out=lb_slots, in_=Bmat.bitcast(F32))
            psL = pps.tile([bp, 256], F32, tag="psL")
            for b in range(bp):
                nc.tensor.matmul(
                    psL,
                    lhsT=Ldiag[:, 65 * b : 65 * b + bp],
                    rhs=M_all[:, b, :],
                    start=(b == 0),
                    stop=(b == bp - 1),
                )
            ltmp = p2w.tile([bp, 256], F32, tag="ltmp")
            lossc = ph2.tile([bp, 1], F32)
            nc.vector.tensor_mul(
                out=ltmp, in0=psL, in1=Amat[:, 0:NV].bitcast(F32)
            )
            nc.vector.tensor_reduce(
                out=lossc, in_=ltmp, axis=mybir.AxisListType.X, op=ALU.add
            )
            nc.sync.dma_start(out=out[:, :], in_=lossc)


_nc_cache: dict = {}


def _numpy_fallback(v, t, v_mask, t_mask, gamma):
    """Exact numpy port of the reference (for non-all-ones masks)."""
    NEG_INF = -1e6
    v = v.astype(np.float32)
    t = t.astype(np.float32)
    vn = v / np.maximum(np.sqrt((v * v).sum(-1, keepdims=True)), 1e-12)
    tn = t / np.maximum(np.sqrt((t * t).sum(-1, keepdims=True)), 1e-12)
    A = np.einsum("bnd,bmd->bnm", vn, tn).astype(np.float32)
    A_raw = A.copy()
    A = np.where(v_mask[:, :, None], A, NEG_INF)
    A = np.where(t_mask[:, None, :], A, NEG_INF)
    Bn = A.shape[0]
    g = np.float32(gamma)
    A_aug = np.concatenate([A, np.full((Bn, NV, 1), g, np.float32)], axis=2)
    A_aug = np.concatenate(
        [A_aug, np.full((Bn, 1, NT + 1), g, np.float32)], axis=1
    )
    v_counts = v_mask.sum(1, keepdims=True) + 1e-9
    mu_real = v_mask.astype(np.float32) / v_counts
    t_counts = t_mask.sum(1, keepdims=True) + 1e-9
    nu_real = t_mask.astype(np.float32) / t_counts
    ones = np.ones((Bn, 1), np.float32)
    mu = np.concatenate([mu_real, ones], 1)
    nu = np.concatenate([nu_real, ones], 1)
    K = A_aug / EPS
    log_mu = np.log(mu + 1e-9)
    log_nu = np.log(nu + 1e-9)
    u = np.zeros_like(mu)
    w = np.zeros_like(nu)

    def lse(x, axis):
        m = x.max(axis=axis, keepdims=True)
        return (m + np.log(np.exp(x - m).sum(axis=axis, keepdims=True))).squeeze(axis)

    for _ in range(ITERS):
        u = log_mu - lse(K + w[:, None, :], 2)
        w = log_nu - lse(K + u[:, :, None], 1)
    T = np.exp(u[:, :, None] + w[:, None, :] + K)
    loss = (T[:, :NV, :NT] * (1.0 - A_raw)).sum((1, 2))
    return np.float32(loss.mean())


def kernel(v, t, v_mask, t_mask, gamma):
    v = np.ascontiguousarray(np.asarray(v), dtype=np.float32)
    t = np.ascontiguousarray(np.asarray(t), dtype=np.float32)
    v_mask = np.asarray(v_mask)
    t_mask = np.asarray(t_mask)
    gamma_f = float(np.asarray(gamma))

    if not (v_mask.all() and t_mask.all()):
        return _numpy_fallback(v, t, v_mask, t_mask, gamma_f)

    try:
        eg = float(np.exp(np.float32(gamma_f) / np.float32(EPS)))
        key = (eg, v.shape, t.shape)
        if key not in _nc_cache:
            _nc_cache[key] = build_bass(eg)
        nc = _nc_cache[key]

        in_maps = [
            {"v": v[i * BP : (i + 1) * BP], "t": t[i * BP : (i + 1) * BP]}
            for i in range(NCORES)
        ]
        res = run_bass_kernel_spmd(nc, in_maps, core_ids=list(range(NCORES)))
        losses = np.concatenate([r["out"][:, 0] for r in res.results])
        return np.float32(np.mean(losses.astype(np.float64)))
    except Exception:
        traceback.print_exc()
        print("kernel: HW path failed, using numpy fallback", file=sys.stderr)
        return _numpy_fallback(v, t, v_mask, t_mask, gamma_f)


if __name__ == "__main__":
    rng = np.random.default_rng(0)
    v = rng.standard_normal((B, NV, D), dtype=np.float32)
    t = rng.standard_normal((B, NT, D), dtype=np.float32)
    vm = np.ones((B, NV), bool)
    tm = np.ones((B, NT), bool)
    print(kernel(v, t, vm, tm, np.float32(0.1)))


# revision 16
# speedup vs baseline: 1.0376x; 1.0376x over previous
"""LocalOTLoss (masked Sinkhorn OT loss) Trainium2 Bass kernel.

Strategy (8 NeuronCores, pure data parallel over batch):
  Each core processes BP=64 batches: v[64,256,512], t[64,128,512] f32.

  Phase 1a (per batch): DMA v/t; ACT Square+accum row sumsq (stored for all
  batches); PE-transpose v/t (12 matmuls); cos-sim matmul psA = t_raw^T
  v_raw in f32r; stash raw psA into the resident X buffer.
  Boundary: ONE Sqrt pass over all 192 sumsq columns + DVE reciprocal
  (batching the Sqrt avoids per-batch ACT table reloads: Sqrt lives in a
  different HW act table than Exp; Square/Copy/Exp share one).
  Phase 1b (per batch): broadcast inv_v row (GpSimd partition_broadcast),
  DVE-fold inv_v, ACT Exp with per-partition inv_t/EPS scale -> X
  (layout [m=NT, b, n=NV+2], dust col NV = e^g, pad col = 0), DVE builds
  M = X*(1-A) in bf16, PE transposes X -> X2 (layout [n, b, c, m]) so the
  w-update becomes a pure matmul.

  Phase 2 (5 Sinkhorn iters, non-log domain): a = mu/(X b) via 64
  accumulating f32r matmuls with a block-one-hot stationary (stride-66
  slot trick, one shared slot buffer); b = nu/(X2^T a) via 128 matmuls
  over X2 (two n-chunks), dustbin handled analytically; transposes of the
  small [64,*] operands ride on the PE.

  Loss = a^T M b per batch via bf16 matmuls -> [64,1]; host averages.

Masks are all-ones in this workload (spec fill=ones); a numpy fallback
handles any other mask pattern.

Toolchain workarounds for this environment:
  - TileContext's final Drain may carry >1 sem wait; this walrus build
    allows only 1 per TPB_CTRL Drain -> _drain_and_barrier patched.
  - Bacc defers register allocation to finalize(); nc.finalize() must run
    before serialization.
  - Custom-DVE ops (tensor_tensor_reduce etc.) crash the runtime here;
    only native DVE/ACT/PE ops are used.
  - f32r matmul operands must be produced by f32r-typed writes (DMA with
    both sides bitcast, or ACT/DVE ops with f32r out). Memset can't write
    f32r -> constants come from inline DRAM tensors.
  - f32r moving operands need an even free dim -> X rows padded to 258.
"""

import sys
import traceback

for _p in ("/opt/trn_rl_repo",):
    if _p not in sys.path:
        sys.path.insert(0, _p)

import numpy as np
import ml_dtypes

import concourse.bass as bass
import concourse.bacc as bacc
import concourse.tile as tile
from concourse import mybir
from concourse.bass_utils import run_bass_kernel_spmd

F32 = mybir.dt.float32
F32R = mybir.dt.float32r
BF16 = mybir.dt.bfloat16
AF = mybir.ActivationFunctionType
ALU = mybir.AluOpType

B, NV, NT, D = 512, 256, 128, 512
NCORES = 8
BP = B // NCORES  # 64 batches per core
EPS = 0.1
ITERS = 5

# effective marginals (mirror reference: exp(log(mu + 1e-9)))
MU_R = 1.0 / (NV + 1e-9) + 1e-9
NU_R = 1.0 / (NT + 1e-9) + 1e-9
DUST = 1.0 + 1e-9


def _install_drain_patch():
    import bass_rust
    from concourse.vector_clock import ScopedClock

    if getattr(tile.TileContext, "_drain_patch_installed", False):
        return

    def _patched_drain_and_barrier(self, tick_clock, wait_clock):
        drain_inst = self.nc.sync.drain()
        wait_clock.add_sem_waits(
            drain_inst.ins, ScopedClock({None: tick_clock.global_clock})
        )
        d = drain_inst.ins
        si = d.sync_info
        waits = list(si.on_wait) if si is not None else []
        if len(waits) > 1:
            si.on_wait = waits[:1]
            d.sync_info = si
            for w in waits[1:]:
                d2 = self.nc.sync.drain().ins
                d2.sync_info = bass_rust.SyncInfo(on_wait=[w], on_update=[])
        self.nc.all_engine_barrier()
        assert self.sems is not None
        popped = self.nc._tile_sem_poison_stack.pop()
        assert popped is self._sem_poison
        self.nc.clear_and_free_semaphores(list(self.sems.allocated().values()))
        self.nc.all_engine_barrier()

    tile.TileContext._drain_and_barrier = _patched_drain_and_barrier
    tile.TileContext._drain_patch_installed = True


_install_drain_patch()


def build_bass(eg: float, bp: int = BP) -> bass.Bass:
    """Build the per-core Bass module. eg = exp(gamma/eps)."""
    nc = bacc.Bacc(trn_type="TRN2")
    v = nc.dram_tensor("v", [bp, NV, D], F32, kind="ExternalInput")
    t = nc.dram_tensor("t", [bp, NT, D], F32, kind="ExternalInput")
    out = nc.dram_tensor("out", [bp, 1], F32, kind="ExternalOutput")
    dram = {
        "ident": nc.inline_tensor(np.eye(128, dtype=np.float32), name="ident"),
        "ones": nc.inline_tensor(np.ones((128, 128), np.float32), name="onesd"),
        "zeros_sl": nc.inline_tensor(
            np.zeros((128, 66 * bp), np.float32), name="zeros_sl"
        ),
        "zeros_bf": nc.inline_tensor(
            np.zeros((128, 66 * bp), ml_dtypes.bfloat16), name="zeros_bf"
        ),
        "egcol": nc.inline_tensor(
            np.concatenate(
                [
                    np.full((128, bp, 1), eg, np.float32),
                    np.zeros((128, bp, 1), np.float32),
                ],
                axis=2,
            ),
            name="egcol",
        ),
    }

    with tile.TileContext(nc) as tc:
        _body(nc, tc, v, t, out, dram, eg, bp)
    nc.finalize()
    return nc


def _body(nc, tc, v, t, out, dram, eg, bp):
    from contextlib import ExitStack

    NVP = NV + 2  # padded X row

    with ExitStack() as ctx:
        consts = ctx.enter_context(tc.tile_pool(name="consts", bufs=1))
        big = ctx.enter_context(tc.tile_pool(name="big", bufs=1))

        ident_sb = consts.tile([128, 128], F32)
        nc.sync.dma_start(out=ident_sb, in_=dram["ident"][:, :])
        ident_r = consts.tile([128, 128], F32R)
        nc.sync.dma_start(out=ident_r, in_=dram["ident"][:, :].bitcast(F32R))

        # Residents
        X_all = big.tile([128, bp, NVP], F32R)  # [m, b, n] raw A then X
        X2_all = big.tile([128, bp, 2, 128], F32R)  # [n, b, chunk, m]
        M_all = big.tile([128, bp, NV], BF16)  # [m, b, n]
        ssq_all = big.tile([128, 3, bp], F32)  # v0, v1, t sumsq
        inv3 = big.tile([128, 3, bp], F32)
        inv_t10 = big.tile([128, bp], F32)
        neg_it = big.tile([128, bp], F32)
        nc.sync.dma_start(
            out=X_all[:, :, NV : NV + 2],
            in_=dram["egcol"][:, :, :].bitcast(F32R),
        )

        # ---------------- Phase 1a ----------------
        with ExitStack() as p1:
            io = p1.enter_context(tc.tile_pool(name="io", bufs=3))
            work = p1.enter_context(tc.tile_pool(name="work", bufs=3))
            pvt = p1.enter_context(tc.tile_pool(name="pvt", bufs=1, space="PSUM"))
            pa = p1.enter_context(tc.tile_pool(name="pa", bufs=2, space="PSUM"))

            for b in range(bp):
                vt = io.tile([128, 2, D], F32, tag="vt")
                nc.sync.dma_start(
                    out=vt, in_=v[b].rearrange("(h p) d -> p h d", p=128)
                )
                tt = io.tile([128, D], F32, tag="tt")
                nc.sync.dma_start(out=tt, in_=t[b])

                # row sumsq on ACT (Square+accum; stays in exp-family table)
                sqd = work.tile([128, D], F32, tag="sqd")
                nc.scalar.activation(
                    out=sqd, in_=vt[:, 0, :], func=AF.Square,
                    accum_out=ssq_all[:, 0, b : b + 1],
                )
                nc.scalar.activation(
                    out=sqd, in_=vt[:, 1, :], func=AF.Square,
                    accum_out=ssq_all[:, 1, b : b + 1],
                )
                nc.scalar.activation(
                    out=sqd, in_=tt, func=AF.Square,
                    accum_out=ssq_all[:, 2, b : b + 1],
                )

                # transposes (raw data)
                psv0 = pvt.tile([128, 512], F32, tag="psv0")
                psv1 = pvt.tile([128, 512], F32, tag="psv1")
                pst = pvt.tile([128, 512], F32, tag="pst")
                for c in range(4):
                    pdst = psv0 if c < 2 else psv1
                    col0 = 256 * (c % 2)
                    for h in range(2):
                        nc.tensor.transpose(
                            out=pdst[:, col0 + 128 * h : col0 + 128 * (h + 1)],
                            in_=vt[:, h, 128 * c : 128 * (c + 1)],
                            identity=ident_sb,
                        )
                    nc.tensor.transpose(
                        out=pst[:, 128 * c : 128 * (c + 1)],
                        in_=tt[:, 128 * c : 128 * (c + 1)],
                        identity=ident_sb,
                    )

                vT = work.tile([128, 4, 256], F32R, tag="vT")
                nc.vector.tensor_copy(
                    out=vT[:, 0:2, :].rearrange("p a b -> p (a b)"), in_=psv0
                )
                nc.vector.tensor_copy(
                    out=vT[:, 2:4, :].rearrange("p a b -> p (a b)"), in_=psv1
                )
                tT = work.tile([128, 512], F32R, tag="tT")
                nc.scalar.copy(out=tT, in_=pst)

                # cos-sim raw matmul
                psA = pa.tile([128, 256], F32, tag="psA")
                for c in range(4):
                    nc.tensor.matmul(
                        psA,
                        lhsT=tT[:, 128 * c : 128 * (c + 1)],
                        rhs=vT[:, c, :],
                        start=(c == 0),
                        stop=(c == 3),
                    )
                # stash raw A into X slot (f32r write)
                nc.vector.tensor_copy(out=X_all[:, b, 0:NV], in_=psA)

        # ---------------- boundary: batched rsqrt ----------------
        with tc.tile_pool(name="bnd", bufs=1) as bnd:
            rt3 = bnd.tile([128, 3, bp], F32)
            nc.scalar.activation(
                out=rt3.rearrange("p a b -> p (a b)"),
                in_=ssq_all.rearrange("p a b -> p (a b)"),
                func=AF.Sqrt,
            )
            nc.vector.reciprocal(
                out=inv3.rearrange("p a b -> p (a b)"),
                in_=rt3.rearrange("p a b -> p (a b)"),
            )
            nc.vector.tensor_scalar_mul(inv_t10, inv3[:, 2, :], 1.0 / EPS)
            nc.vector.tensor_scalar_mul(neg_it, inv3[:, 2, :], -1.0)

        # ---------------- Phase 1b ----------------
        with ExitStack() as p1b:
            wk = p1b.enter_context(tc.tile_pool(name="wk", bufs=3))
            pr = p1b.enter_context(tc.tile_pool(name="pr", bufs=2, space="PSUM"))
            px = p1b.enter_context(tc.tile_pool(name="px", bufs=2, space="PSUM"))

            for b in range(bp):
                psr = pr.tile([1, 256], F32, tag="psr")
                nc.tensor.transpose(
                    out=psr[:, 0:128], in_=inv3[:, 0, b : b + 1], identity=ident_sb
                )
                nc.tensor.transpose(
                    out=psr[:, 128:256], in_=inv3[:, 1, b : b + 1], identity=ident_sb
                )
                ivrow = wk.tile([1, 256], F32, tag="ivrow")
                nc.vector.tensor_copy(out=ivrow, in_=psr)
                ivb = wk.tile([128, 256], F32, tag="ivb")
                nc.gpsimd.partition_broadcast(ivb, ivrow)

                An = wk.tile([128, 256], F32, tag="An")
                nc.vector.tensor_mul(
                    out=An, in0=X_all[:, b, 0:NV].bitcast(F32), in1=ivb
                )
                # X = exp(An * inv_t / EPS)
                nc.scalar.activation(
                    out=X_all[:, b, 0:NV], in_=An, func=AF.Exp,
                    scale=inv_t10[:, b : b + 1],
                )
                # om = 1 - An*inv_t
                om = wk.tile([128, 256], F32, tag="om")
                nc.vector.tensor_scalar(
                    out=om, in0=An, scalar1=neg_it[:, b : b + 1], scalar2=1.0,
                    op0=ALU.mult, op1=ALU.add,
                )
                nc.vector.tensor_mul(
                    out=M_all[:, b, :], in0=om, in1=X_all[:, b, 0:NV].bitcast(F32)
                )
                # X2 = X^T (two 128-chunks)
                psX2 = px.tile([128, 256], F32R, tag="psX2")
                nc.tensor.transpose(
                    out=psX2[:, 0:128], in_=X_all[:, b, 0:128], identity=ident_r
                )
                nc.tensor.transpose(
                    out=psX2[:, 128:256], in_=X_all[:, b, 128:256], identity=ident_r
                )
                nc.vector.tensor_copy(
                    out=X2_all[:, b, :, :].rearrange("p a b -> p (a b)"),
                    in_=psX2.bitcast(F32),
                )

        # ---------------- Phase 2: Sinkhorn ----------------
        with ExitStack() as p2:
            ph2 = p2.enter_context(tc.tile_pool(name="ph2", bufs=1))
            p2w = p2.enter_context(tc.tile_pool(name="p2w", bufs=3))
            pps = p2.enter_context(tc.tile_pool(name="pps", bufs=1, space="PSUM"))
            ppw = p2.enter_context(tc.tile_pool(name="ppw", bufs=1, space="PSUM"))
            ppt = p2.enter_context(tc.tile_pool(name="ppt", bufs=2, space="PSUM"))

            Bmat = ph2.tile([128, bp], F32R)
            nc.sync.dma_start(out=Bmat, in_=dram["ones"][:, 0:bp].bitcast(F32R))
            bdust = ph2.tile([bp, 1], F32)
            nc.vector.memset(bdust, 1.0)
            Sdiag = ph2.tile([128, 66 * bp], F32R)  # shared slot buffer
            nc.sync.dma_start(out=Sdiag, in_=dram["zeros_sl"][:, :].bitcast(F32R))
            Ldiag = ph2.tile([128, 66 * bp], BF16)
            nc.sync.dma_start(out=Ldiag, in_=dram["zeros_bf"][:, :].bitcast(BF16))
            Amat = ph2.tile([bp, NVP], F32R)

            sl_slots = bass.AP(
                tensor=Sdiag.tensor,
                offset=Sdiag.offset,
                ap=[list(Sdiag.ap[0]), [66, bp]],
            )
            lb_slots = bass.AP(
                tensor=Ldiag.tensor,
                offset=Ldiag.offset,
                ap=[list(Ldiag.ap[0]), [66, bp]],
            )

            for it in range(ITERS):
                # -- u-update: a = mu / (X b); dust col gives eg*sum(b) --
                nc.vector.tensor_copy(out=sl_slots, in_=Bmat.bitcast(F32))
                psS = pps.tile([bp, NVP], F32, tag="psS")
                for b in range(bp):
                    nc.tensor.matmul(
                        psS,
                        lhsT=Sdiag[:, 65 * b : 65 * b + bp],
                        rhs=X_all[:, b, :],
                        start=(b == 0),
                        stop=(b == bp - 1),
                    )
                bd_eg = p2w.tile([bp, 1], F32, tag="bd_eg")
                nc.vector.tensor_scalar_mul(bd_eg, bdust, eg)
                den = p2w.tile([bp, NVP], F32, tag="den")
                nc.vector.tensor_scalar(
                    out=den, in0=psS, scalar1=bd_eg, scalar2=None, op0=ALU.add
                )
                rec = p2w.tile([bp, NVP], F32, tag="rec")
                nc.vector.reciprocal(out=rec, in_=den)
                nc.vector.tensor_scalar_mul(Amat[:, 0:NV], rec[:, 0:NV], MU_R)
                nc.vector.tensor_scalar_mul(
                    Amat[:, NV:NVP], rec[:, NV:NVP], DUST
                )

                # -- w-update: b = nu / (X2 a + eg*a_dust) --
                psT = ppt.tile([128, 128], F32R, tag="psT")
                nc.tensor.transpose(
                    out=psT[:, 0:bp], in_=Amat[:, 0:128],
                    identity=ident_r[0:bp, 0:bp],
                )
                nc.tensor.transpose(
                    out=psT[:, bp : 2 * bp], in_=Amat[:, 128:256],
                    identity=ident_r[0:bp, 0:bp],
                )
                psW = ppw.tile([bp, 128], F32, tag="psW")
                for c in range(2):
                    nc.vector.tensor_copy(
                        out=sl_slots, in_=psT[:, bp * c : bp * (c + 1)].bitcast(F32)
                    )
                    for b in range(bp):
                        nc.tensor.matmul(
                            psW,
                            lhsT=Sdiag[:, 65 * b : 65 * b + bp],
                            rhs=X2_all[:, b, c, :],
                            start=(c == 0 and b == 0),
                            stop=(c == 1 and b == bp - 1),
                        )
                ad_eg = p2w.tile([bp, 1], F32, tag="ad_eg")
                nc.vector.tensor_scalar_mul(
                    ad_eg, Amat[:, NV : NV + 1].bitcast(F32), eg
                )
                wden = p2w.tile([bp, 128], F32, tag="wden")
                nc.vector.tensor_scalar(
                    out=wden, in0=psW, scalar1=ad_eg, scalar2=None, op0=ALU.add
                )
                wrec = p2w.tile([bp, 128], F32R, tag="wrec")
                with nc.allow_low_precision(reason="f32r feed for PE transpose"):
                    nc.vector.reciprocal(out=wrec, in_=wden)
                psB = ppt.tile([128, bp], F32R, tag="psB")
                nc.tensor.transpose(out=psB, in_=wrec, identity=ident_r[0:bp, 0:bp])
                nc.vector.tensor_scalar_mul(Bmat, psB.bitcast(F32), NU_R)

                # b_dust = DUST / (eg * sum_n a_total)
                sa = p2w.tile([bp, 1], F32, tag="sa")
                nc.vector.tensor_reduce(
                    out=sa, in_=Amat[:, 0 : NV + 1].bitcast(F32),
                    axis=mybir.AxisListType.X, op=ALU.add,
                )
                sa2 = p2w.tile([bp, 1], F32, tag="sa2")
                nc.vector.tensor_scalar_mul(sa2, sa, eg)
                sa3 = p2w.tile([bp, 1], F32, tag="sa3")
                nc.vector.reciprocal(out=sa3, in_=sa2)
                nc.vector.tensor_scalar_mul(bdust, sa3, DUST)

            # -- loss = a^T M b per batch (bf16 matmuls) --
            nc.vector.tensor_copy(out=lb_slots, in_=Bmat.bitcast(F32))
            psL = pps.tile([bp, 256], F32, tag="psL")
            for b in range(bp):
                nc.tensor.matmul(
                    psL,
                    lhsT=Ldiag[:, 65 * b : 65 * b + bp],
                    rhs=M_all[:, b, :],
                    start=(b == 0),
                    stop=(b == bp - 1),
                )
            ltmp = p2w.tile([bp, 256], F32, tag="ltmp")
            lossc = ph2.tile([bp, 1], F32)
            nc.vector.tensor_mul(
                out=ltmp, in0=psL, in1=Amat[:, 0:NV].bitcast(F32)
            )
            nc.vector.tensor_reduce(
                out=lossc, in_=ltmp, axis=mybir.AxisListType.X, op=ALU.add
            )
            nc.sync.dma_start(out=out[:, :], in_=lossc)


_nc_cache: dict = {}


def _numpy_fallback(v, t, v_mask, t_mask, gamma):
    """Exact numpy port of the reference (for non-all-ones masks)."""
    NEG_INF = -1e6
    v = v.astype(np.float32)
    t = t.astype(np.float32)
    vn = v / np.maximum(np.sqrt((v * v).sum(-1, keepdims=True)), 1e-12)
    tn = t / np.maximum(np.sqrt((t * t).sum(-1, keepdims=True)), 1e-12)
    A = np.einsum("bnd,bmd->bnm", vn, tn).astype(np.float32)
    A_raw = A.copy()
    A = np.where(v_mask[:, :, None], A, NEG_INF)
    A = np.where(t_mask[:, None, :], A, NEG_INF)
    Bn = A.shape[0]
    g = np.float32(gamma)
    A_aug = np.concatenate([A, np.full((Bn, NV, 1), g, np.float32)], axis=2)
    A_aug = np.concatenate(
        [A_aug, np.full((Bn, 1, NT + 1), g, np.float32)], axis=1
    )
    v_counts = v_mask.sum(1, keepdims=True) + 1e-9
    mu_real = v_mask.astype(np.float32) / v_counts
    t_counts = t_mask.sum(1, keepdims=True) + 1e-9
    nu_real = t_mask.astype(np.float32) / t_counts
    ones = np.ones((Bn, 1), np.float32)
    mu = np.concatenate([mu_real, ones], 1)
    nu = np.concatenate([nu_real, ones], 1)
    K = A_aug / EPS
    log_mu = np.log(mu + 1e-9)
    log_nu = np.log(nu + 1e-9)
    u = np.zeros_like(mu)
    w = np.zeros_like(nu)

    def lse(x, axis):
        m = x.max(axis=axis, keepdims=True)
        return (m + np.log(np.exp(x - m).sum(axis=axis, keepdims=True))).squeeze(axis)

    for _ in range(ITERS):
        u = log_mu - lse(K + w[:, None, :], 2)
        w = log_nu - lse(K + u[:, :, None], 1)
    T = np.exp(u[:, :, None] + w[:, None, :] + K)
    loss = (T[:, :NV, :NT] * (1.0 - A_raw)).sum((1, 2))
    return np.float32(loss.mean())


def kernel(v, t, v_mask, t_mask, gamma):
    v = np.ascontiguousarray(np.asarray(v), dtype=np.float32)
    t = np.ascontiguousarray(np.asarray(t), dtype=np.float32)
    v_mask = np.asarray(v_mask)
    t_mask = np.asarray(t_mask)
    gamma_f = float(np.asarray(gamma))

    if not (v_mask.all() and t_mask.all()):
        return _numpy_fallback(v, t, v_mask, t_mask, gamma_f)

    try:
        eg = float(np.exp(np.float32(gamma_f) / np.float32(EPS)))
        key = (eg, v.shape, t.shape)
        if key not in _nc_cache:
            _nc_cache[key] = build_bass(eg)
        nc = _nc_cache[key]

        in_maps = [
            {"v": v[i * BP : (i + 1) * BP], "t": t[i * BP : (i + 1) * BP]}
            for i in range(NCORES)
        ]
        res = run_bass_kernel_spmd(nc, in_maps, core_ids=list(range(NCORES)))
        losses = np.concatenate([r["out"][:, 0] for r in res.results])
        return np.float32(np.mean(losses.astype(np.float64)))
    except Exception:
        traceback.print_exc()
        print("kernel: HW path failed, using numpy fallback", file=sys.stderr)
        return _numpy_fallback(v, t, v_mask, t_mask, gamma_f)


if __name__ == "__main__":
    rng = np.random.default_rng(0)
    v = rng.standard_normal((B, NV, D), dtype=np.float32)
    t = rng.standard_normal((B, NT, D), dtype=np.float32)
    vm = np.ones((B, NV), bool)
    tm = np.ones((B, NT), bool)
    print(kernel(v, t, vm, tm, np.float32(0.1)))


# revision 18
# speedup vs baseline: 1.0829x; 1.0436x over previous
"""LocalOTLoss (masked Sinkhorn OT loss) Trainium2 Bass kernel.

Strategy (8 NeuronCores, pure data parallel over batch):
  Each core processes BP=64 batches: v[64,256,512], t[64,128,512] f32.

  Phase 1a (per batch): DMA v/t; ACT Square+accum row sumsq (stored for all
  batches); PE-transpose v/t (12 matmuls); cos-sim matmul psA = t_raw^T
  v_raw in f32r; stash raw psA into the resident X buffer.
  Boundary: ONE Sqrt pass over all 192 sumsq columns + DVE reciprocal
  (batching the Sqrt avoids per-batch ACT table reloads: Sqrt lives in a
  different HW act table than Exp; Square/Copy/Exp share one).
  Phase 1b (per batch): broadcast inv_v row (GpSimd partition_broadcast),
  DVE-fold inv_v, ACT Exp with per-partition inv_t/EPS scale -> X
  (layout [m=NT, b, n=NV+2], dust col NV = e^g, pad col = 0), DVE builds
  M = X*(1-A) in bf16, PE transposes X -> X2 (layout [n, b, c, m]) so the
  w-update becomes a pure matmul.

  Phase 2 (5 Sinkhorn iters, non-log domain): a = mu/(X b) via 64
  accumulating f32r matmuls with a block-one-hot stationary (stride-66
  slot trick, one shared slot buffer); b = nu/(X2^T a) via 128 matmuls
  over X2 (two n-chunks), dustbin handled analytically; transposes of the
  small [64,*] operands ride on the PE.

  Loss = a^T M b per batch via bf16 matmuls -> [64,1]; host averages.

Masks are all-ones in this workload (spec fill=ones); a numpy fallback
handles any other mask pattern.

Toolchain workarounds for this environment:
  - TileContext's final Drain may carry >1 sem wait; this walrus build
    allows only 1 per TPB_CTRL Drain -> _drain_and_barrier patched.
  - Bacc defers register allocation to finalize(); nc.finalize() must run
    before serialization.
  - Custom-DVE ops (tensor_tensor_reduce etc.) crash the runtime here;
    only native DVE/ACT/PE ops are used.
  - f32r matmul operands must be produced by f32r-typed writes (DMA with
    both sides bitcast, or ACT/DVE ops with f32r out). Memset can't write
    f32r -> constants come from inline DRAM tensors.
  - f32r moving operands need an even free dim -> X rows padded to 258.
"""

import sys
import traceback

for _p in ("/opt/trn_rl_repo",):
    if _p not in sys.path:
        sys.path.insert(0, _p)

import numpy as np
import ml_dtypes

import concourse.bass as bass
import concourse.bacc as bacc
import concourse.tile as tile
from concourse import mybir
from concourse.bass_utils import run_bass_kernel_spmd

F32 = mybir.dt.float32
F32R = mybir.dt.float32r
BF16 = mybir.dt.bfloat16
AF = mybir.ActivationFunctionType
ALU = mybir.AluOpType

B, NV, NT, D = 512, 256, 128, 512
NCORES = 8
BP = B // NCORES  # 64 batches per core
EPS = 0.1
ITERS = 5

# effective marginals (mirror reference: exp(log(mu + 1e-9)))
MU_R = 1.0 / (NV + 1e-9) + 1e-9
NU_R = 1.0 / (NT + 1e-9) + 1e-9
DUST = 1.0 + 1e-9


def _install_drain_patch():
    import bass_rust
    from concourse.vector_clock import ScopedClock

    if getattr(tile.TileContext, "_drain_patch_installed", False):
        return

    def _patched_drain_and_barrier(self, tick_clock, wait_clock):
        drain_inst = self.nc.sync.drain()
        wait_clock.add_sem_waits(
            drain_inst.ins, ScopedClock({None: tick_clock.global_clock})
        )
        d = drain_inst.ins
        si = d.sync_info
        waits = list(si.on_wait) if si is not None else []
        if len(waits) > 1:
            si.on_wait = waits[:1]
            d.sync_info = si
            for w in waits[1:]:
                d2 = self.nc.sync.drain().ins
                d2.sync_info = bass_rust.SyncInfo(on_wait=[w], on_update=[])
        self.nc.all_engine_barrier()
        assert self.sems is not None
        popped = self.nc._tile_sem_poison_stack.pop()
        assert popped is self._sem_poison
        self.nc.clear_and_free_semaphores(list(self.sems.allocated().values()))
        self.nc.all_engine_barrier()

    tile.TileContext._drain_and_barrier = _patched_drain_and_barrier
    tile.TileContext._drain_patch_installed = True


_install_drain_patch()


def build_bass(eg: float, bp: int = BP) -> bass.Bass:
    """Build the per-core Bass module. eg = exp(gamma/eps)."""
    nc = bacc.Bacc(trn_type="TRN2")
    v = nc.dram_tensor("v", [bp, NV, D], F32, kind="ExternalInput")
    t = nc.dram_tensor("t", [bp, NT, D], F32, kind="ExternalInput")
    out = nc.dram_tensor("out", [bp, 1], F32, kind="ExternalOutput")
    dram = {
        "ident": nc.inline_tensor(np.eye(128, dtype=np.float32), name="ident"),
        "ones": nc.inline_tensor(np.ones((128, 128), np.float32), name="onesd"),
        "zeros_sl": nc.inline_tensor(
            np.zeros((128, 66 * bp), np.float32), name="zeros_sl"
        ),
        "zeros_bf": nc.inline_tensor(
            np.zeros((128, 66 * bp), ml_dtypes.bfloat16), name="zeros_bf"
        ),
        "egcol": nc.inline_tensor(
            np.concatenate(
                [
                    np.full((128, bp, 1), eg, np.float32),
                    np.zeros((128, bp, 1), np.float32),
                ],
                axis=2,
            ),
            name="egcol",
        ),
    }

    with tile.TileContext(nc) as tc:
        _body(nc, tc, v, t, out, dram, eg, bp)
    nc.finalize()
    return nc


def _body(nc, tc, v, t, out, dram, eg, bp):
    from contextlib import ExitStack

    NVP = NV + 2  # padded X row

    with ExitStack() as ctx:
        consts = ctx.enter_context(tc.tile_pool(name="consts", bufs=1))
        big = ctx.enter_context(tc.tile_pool(name="big", bufs=1))

        ident_sb = consts.tile([128, 128], F32)
        nc.sync.dma_start(out=ident_sb, in_=dram["ident"][:, :])
        ident_r = consts.tile([128, 128], F32R)
        nc.sync.dma_start(out=ident_r, in_=dram["ident"][:, :].bitcast(F32R))

        # Residents
        X_all = big.tile([128, bp, NVP], F32R)  # [m, b, n] raw A then X
        X2_all = big.tile([128, bp, 2, 128], F32R)  # [n, b, chunk, m]
        M_all = big.tile([128, bp, NV], BF16)  # [m, b, n]
        ssq_all = big.tile([128, 3, bp], F32)  # v0, v1, t sumsq
        invrows = big.tile([2 * bp, 128], F32)  # row j=h*bp+b: inv_v half h
        inv_t10 = big.tile([128, bp], F32)
        neg_it = big.tile([128, bp], F32)
        nc.sync.dma_start(
            out=X_all[:, :, NV : NV + 2],
            in_=dram["egcol"][:, :, :].bitcast(F32R),
        )

        # ---------------- Phase 1a ----------------
        with ExitStack() as p1:
            io = p1.enter_context(tc.tile_pool(name="io", bufs=3))
            work = p1.enter_context(tc.tile_pool(name="work", bufs=3))
            pvt = p1.enter_context(tc.tile_pool(name="pvt", bufs=1, space="PSUM"))
            pa = p1.enter_context(tc.tile_pool(name="pa", bufs=2, space="PSUM"))

            for b in range(bp):
                vt = io.tile([128, 2, D], F32, tag="vt")
                nc.sync.dma_start(
                    out=vt, in_=v[b].rearrange("(h p) d -> p h d", p=128)
                )
                tt = io.tile([128, D], F32, tag="tt")
                nc.sync.dma_start(out=tt, in_=t[b])

                # row sumsq on ACT (Square+accum; stays in exp-family table)
                sqd = work.tile([128, D], F32, tag="sqd")
                nc.scalar.activation(
                    out=sqd, in_=vt[:, 0, :], func=AF.Square,
                    accum_out=ssq_all[:, 0, b : b + 1],
                )
                nc.scalar.activation(
                    out=sqd, in_=vt[:, 1, :], func=AF.Square,
                    accum_out=ssq_all[:, 1, b : b + 1],
                )
                nc.scalar.activation(
                    out=sqd, in_=tt, func=AF.Square,
                    accum_out=ssq_all[:, 2, b : b + 1],
                )

                # transposes (raw data)
                psv0 = pvt.tile([128, 512], F32, tag="psv0")
                psv1 = pvt.tile([128, 512], F32, tag="psv1")
                pst = pvt.tile([128, 512], F32, tag="pst")
                for c in range(4):
                    pdst = psv0 if c < 2 else psv1
                    col0 = 256 * (c % 2)
                    for h in range(2):
                        nc.tensor.transpose(
                            out=pdst[:, col0 + 128 * h : col0 + 128 * (h + 1)],
                            in_=vt[:, h, 128 * c : 128 * (c + 1)],
                            identity=ident_sb,
                        )
                    nc.tensor.transpose(
                        out=pst[:, 128 * c : 128 * (c + 1)],
                        in_=tt[:, 128 * c : 128 * (c + 1)],
                        identity=ident_sb,
                    )

                vT = work.tile([128, 4, 256], F32R, tag="vT")
                nc.vector.tensor_copy(
                    out=vT[:, 0:2, :].rearrange("p a b -> p (a b)"), in_=psv0
                )
                nc.vector.tensor_copy(
                    out=vT[:, 2:4, :].rearrange("p a b -> p (a b)"), in_=psv1
                )
                tT = work.tile([128, 512], F32R, tag="tT")
                nc.scalar.copy(out=tT, in_=pst)

                # cos-sim raw matmul
                psA = pa.tile([128, 256], F32, tag="psA")
                for c in range(4):
                    nc.tensor.matmul(
                        psA,
                        lhsT=tT[:, 128 * c : 128 * (c + 1)],
                        rhs=vT[:, c, :],
                        start=(c == 0),
                        stop=(c == 3),
                    )
                # stash raw A into X slot (f32r write)
                nc.vector.tensor_copy(out=X_all[:, b, 0:NV], in_=psA)

        # ---------------- boundary: batched rsqrt ----------------
        # v-norms transposed to row layout (one PE transpose for ALL
        # batches) so phase 1b needs no per-batch psr transposes.
        with tc.tile_pool(name="bnd", bufs=1) as bnd, \
             tc.tile_pool(name="bndp", bufs=1, space="PSUM") as bndp:
            psq = bndp.tile([2 * bp, 128], F32)
            nc.tensor.transpose(
                out=psq,
                in_=ssq_all[:, 0:2, :].rearrange("p a b -> p (a b)"),
                identity=ident_sb,
            )
            rows_rt = bnd.tile([2 * bp, 128], F32)
            nc.scalar.activation(out=rows_rt, in_=psq, func=AF.Sqrt)
            nc.vector.reciprocal(out=invrows, in_=rows_rt)
            rtT = bnd.tile([128, bp], F32)
            nc.scalar.activation(out=rtT, in_=ssq_all[:, 2, :], func=AF.Sqrt)
            invT = bnd.tile([128, bp], F32)
            nc.vector.reciprocal(out=invT, in_=rtT)
            nc.vector.tensor_scalar_mul(inv_t10, invT, 1.0 / EPS)
            nc.vector.tensor_scalar_mul(neg_it, invT, -1.0)

        # ---------------- Phase 1b ----------------
        with ExitStack() as p1b:
            wk = p1b.enter_context(tc.tile_pool(name="wk", bufs=3))
            px = p1b.enter_context(tc.tile_pool(name="px", bufs=2, space="PSUM"))

            for b in range(bp):
                ivrow = wk.tile([1, 2, 128], F32, tag="ivrow")
                nc.sync.dma_start(out=ivrow[:, 0, :], in_=invrows[b : b + 1, :])
                nc.sync.dma_start(
                    out=ivrow[:, 1, :], in_=invrows[bp + b : bp + b + 1, :]
                )
                ivb = wk.tile([128, 256], F32, tag="ivb")
                nc.gpsimd.partition_broadcast(
                    ivb, ivrow.rearrange("p a b -> p (a b)")
                )

                An = wk.tile([128, 256], F32, tag="An")
                nc.vector.tensor_mul(
                    out=An, in0=X_all[:, b, 0:NV].bitcast(F32), in1=ivb
                )
                # X = exp(An * inv_t / EPS)
                nc.scalar.activation(
                    out=X_all[:, b, 0:NV], in_=An, func=AF.Exp,
                    scale=inv_t10[:, b : b + 1],
                )
                # om = 1 - An*inv_t
                om = wk.tile([128, 256], F32, tag="om")
                nc.vector.tensor_scalar(
                    out=om, in0=An, scalar1=neg_it[:, b : b + 1], scalar2=1.0,
                    op0=ALU.mult, op1=ALU.add,
                )
                nc.vector.tensor_mul(
                    out=M_all[:, b, :], in0=om, in1=X_all[:, b, 0:NV].bitcast(F32)
                )
                # X2 = X^T (two 128-chunks)
                psX2 = px.tile([128, 256], F32R, tag="psX2")
                nc.tensor.transpose(
                    out=psX2[:, 0:128], in_=X_all[:, b, 0:128], identity=ident_r
                )
                nc.tensor.transpose(
                    out=psX2[:, 128:256], in_=X_all[:, b, 128:256], identity=ident_r
                )
                nc.vector.tensor_copy(
                    out=X2_all[:, b, :, :].rearrange("p a b -> p (a b)"),
                    in_=psX2.bitcast(F32),
                )

        # ---------------- Phase 2: Sinkhorn ----------------
        with ExitStack() as p2:
            ph2 = p2.enter_context(tc.tile_pool(name="ph2", bufs=1))
            p2w = p2.enter_context(tc.tile_pool(name="p2w", bufs=3))
            pps = p2.enter_context(tc.tile_pool(name="pps", bufs=1, space="PSUM"))
            ppw = p2.enter_context(tc.tile_pool(name="ppw", bufs=1, space="PSUM"))
            ppt = p2.enter_context(tc.tile_pool(name="ppt", bufs=2, space="PSUM"))

            Bmat = ph2.tile([128, bp], F32R)
            nc.sync.dma_start(out=Bmat, in_=dram["ones"][:, 0:bp].bitcast(F32R))
            bdust = ph2.tile([bp, 1], F32)
            nc.vector.memset(bdust, 1.0)
            Sdiag = ph2.tile([128, 66 * bp], F32R)  # shared slot buffer
            nc.sync.dma_start(out=Sdiag, in_=dram["zeros_sl"][:, :].bitcast(F32R))
            Ldiag = ph2.tile([128, 66 * bp], BF16)
            nc.sync.dma_start(out=Ldiag, in_=dram["zeros_bf"][:, :].bitcast(BF16))
            Amat = ph2.tile([bp, NVP], F32R)

            sl_slots = bass.AP(
                tensor=Sdiag.tensor,
                offset=Sdiag.offset,
                ap=[list(Sdiag.ap[0]), [66, bp]],
            )
            lb_slots = bass.AP(
                tensor=Ldiag.tensor,
                offset=Ldiag.offset,
                ap=[list(Ldiag.ap[0]), [66, bp]],
            )

            for it in range(ITERS):
                # -- u-update: a = mu / (X b); dust col gives eg*sum(b) --
                nc.vector.tensor_copy(out=sl_slots, in_=Bmat.bitcast(F32))
                psS = pps.tile([bp, NVP], F32, tag="psS")
                for b in range(bp):
                    nc.tensor.matmul(
                        psS,
                        lhsT=Sdiag[:, 65 * b : 65 * b + bp],
                        rhs=X_all[:, b, :],
                        start=(b == 0),
                        stop=(b == bp - 1),
                    )
                bd_eg = p2w.tile([bp, 1], F32, tag="bd_eg")
                nc.vector.tensor_scalar_mul(bd_eg, bdust, eg)
                den = p2w.tile([bp, NVP], F32, tag="den")
                nc.vector.tensor_scalar(
                    out=den, in0=psS, scalar1=bd_eg, scalar2=None, op0=ALU.add
                )
                rec = p2w.tile([bp, NVP], F32, tag="rec")
                nc.vector.reciprocal(out=rec, in_=den)
                nc.vector.tensor_scalar_mul(Amat[:, 0:NV], rec[:, 0:NV], MU_R)
                nc.vector.tensor_scalar_mul(
                    Amat[:, NV:NVP], rec[:, NV:NVP], DUST
                )

                # -- w-update: b = nu / (X2 a + eg*a_dust) --
                psT = ppt.tile([128, 128], F32R, tag="psT")
                nc.tensor.transpose(
                    out=psT[:, 0:bp], in_=Amat[:, 0:128],
                    identity=ident_r[0:bp, 0:bp],
                )
                nc.tensor.transpose(
                    out=psT[:, bp : 2 * bp], in_=Amat[:, 128:256],
                    identity=ident_r[0:bp, 0:bp],
                )
                psW = ppw.tile([bp, 128], F32, tag="psW")
                for c in range(2):
                    nc.vector.tensor_copy(
                        out=sl_slots, in_=psT[:, bp * c : bp * (c + 1)].bitcast(F32)
                    )
                    for b in range(bp):
                        nc.tensor.matmul(
                            psW,
                            lhsT=Sdiag[:, 65 * b : 65 * b + bp],
                            rhs=X2_all[:, b, c, :],
                            start=(c == 0 and b == 0),
                            stop=(c == 1 and b == bp - 1),
                        )
                ad_eg = p2w.tile([bp, 1], F32, tag="ad_eg")
                nc.vector.tensor_scalar_mul(
                    ad_eg, Amat[:, NV : NV + 1].bitcast(F32), eg
                )
                wden = p2w.tile([bp, 128], F32, tag="wden")
                nc.vector.tensor_scalar(
                    out=wden, in0=psW, scalar1=ad_eg, scalar2=None, op0=ALU.add
                )
                wrec = p2w.tile([bp, 128], F32R, tag="wrec")
                with nc.allow_low_precision(reason="f32r feed for PE transpose"):
                    nc.vector.reciprocal(out=wrec, in_=wden)
                psB = ppt.tile([128, bp], F32R, tag="psB")
                nc.tensor.transpose(out=psB, in_=wrec, identity=ident_r[0:bp, 0:bp])
                nc.vector.tensor_scalar_mul(Bmat, psB.bitcast(F32), NU_R)

                # b_dust = DUST / (eg * sum_n a_total)
                sa = p2w.tile([bp, 1], F32, tag="sa")
                nc.vector.tensor_reduce(
                    out=sa, in_=Amat[:, 0 : NV + 1].bitcast(F32),
                    axis=mybir.AxisListType.X, op=ALU.add,
                )
                sa2 = p2w.tile([bp, 1], F32, tag="sa2")
                nc.vector.tensor_scalar_mul(sa2, sa, eg)
                sa3 = p2w.tile([bp, 1], F32, tag="sa3")
                nc.vector.reciprocal(out=sa3, in_=sa2)
                nc.vector.tensor_scalar_mul(bdust, sa3, DUST)

            # -- loss = a^T M b per batch (bf16 matmuls) --
            nc.vector.tensor_copy(out=lb_slots, in_=Bmat.bitcast(F32))
            psL = pps.tile([bp, 256], F32, tag="psL")
            for b in range(bp):
                nc.tensor.matmul(
                    psL,
                    lhsT=Ldiag[:, 65 * b : 65 * b + bp],
                    rhs=M_all[:, b, :],
                    start=(b == 0),
                    stop=(b == bp - 1),
                )
            ltmp = p2w.tile([bp, 256], F32, tag="ltmp")
            lossc = ph2.tile([bp, 1], F32)
            nc.vector.tensor_mul(
                out=ltmp, in0=psL, in1=Amat[:, 0:NV].bitcast(F32)
            )
            nc.vector.tensor_reduce(
                out=lossc, in_=ltmp, axis=mybir.AxisListType.X, op=ALU.add
            )
            nc.sync.dma_start(out=out[:, :], in_=lossc)


_nc_cache: dict = {}


def _numpy_fallback(v, t, v_mask, t_mask, gamma):
    """Exact numpy port of the reference (for non-all-ones masks)."""
    NEG_INF = -1e6
    v = v.astype(np.float32)
    t = t.astype(np.float32)
    vn = v / np.maximum(np.sqrt((v * v).sum(-1, keepdims=True)), 1e-12)
    tn = t / np.maximum(np.sqrt((t * t).sum(-1, keepdims=True)), 1e-12)
    A = np.einsum("bnd,bmd->bnm", vn, tn).astype(np.float32)
    A_raw = A.copy()
    A = np.where(v_mask[:, :, None], A, NEG_INF)
    A = np.where(t_mask[:, None, :], A, NEG_INF)
    Bn = A.shape[0]
    g = np.float32(gamma)
    A_aug = np.concatenate([A, np.full((Bn, NV, 1), g, np.float32)], axis=2)
    A_aug = np.concatenate(
        [A_aug, np.full((Bn, 1, NT + 1), g, np.float32)], axis=1
    )
    v_counts = v_mask.sum(1, keepdims=True) + 1e-9
    mu_real = v_mask.astype(np.float32) / v_counts
    t_counts = t_mask.sum(1, keepdims=True) + 1e-9
    nu_real = t_mask.astype(np.float32) / t_counts
    ones = np.ones((Bn, 1), np.float32)
    mu = np.concatenate([mu_real, ones], 1)
    nu = np.concatenate([nu_real, ones], 1)
    K = A_aug / EPS
    log_mu = np.log(mu + 1e-9)
    log_nu = np.log(nu + 1e-9)
    u = np.zeros_like(mu)
    w = np.zeros_like(nu)

    def lse(x, axis):
        m = x.max(axis=axis, keepdims=True)
        return (m + np.log(np.exp(x - m).sum(axis=axis, keepdims=True))).squeeze(axis)

    for _ in range(ITERS):
        u = log_mu - lse(K + w[:, None, :], 2)
        w = log_nu - lse(K + u[:, :, None], 1)
    T = np.exp(u[:, :, None] + w[:, None, :] + K)
    loss = (T[:, :NV, :NT] * (1.0 - A_raw)).sum((1, 2))
    return np.float32(loss.mean())


def kernel(v, t, v_mask, t_mask, gamma):
    v = np.ascontiguousarray(np.asarray(v), dtype=np.float32)
    t = np.ascontiguousarray(np.asarray(t), dtype=np.float32)
    v_mask = np.asarray(v_mask)
    t_mask = np.asarray(t_mask)
    gamma_f = float(np.asarray(gamma))

    if not (v_mask.all() and t_mask.all()):
        return _numpy_fallback(v, t, v_mask, t_mask, gamma_f)

    try:
        eg = float(np.exp(np.float32(gamma_f) / np.float32(EPS)))
        key = (eg, v.shape, t.shape)
        if key not in _nc_cache:
            _nc_cache[key] = build_bass(eg)
        nc = _nc_cache[key]

        in_maps = [
            {"v": v[i * BP : (i + 1) * BP], "t": t[i * BP : (i + 1) * BP]}
            for i in range(NCORES)
        ]
        res = run_bass_kernel_spmd(nc, in_maps, core_ids=list(range(NCORES)))
        losses = np.concatenate([r["out"][:, 0] for r in res.results])
        return np.float32(np.mean(losses.astype(np.float64)))
    except Exception:
        traceback.print_exc()
        print("kernel: HW path failed, using numpy fallback", file=sys.stderr)
        return _numpy_fallback(v, t, v_mask, t_mask, gamma_f)


if __name__ == "__main__":
    rng = np.random.default_rng(0)
    v = rng.standard_normal((B, NV, D), dtype=np.float32)
    t = rng.standard_normal((B, NT, D), dtype=np.float32)
    vm = np.ones((B, NV), bool)
    tm = np.ones((B, NT), bool)
    print(kernel(v, t, vm, tm, np.float32(0.1)))
